# revision 1
# baseline (speedup 1.0000x reference)
"""Sobel gradient magnitude kernel for Trainium2 (8 NeuronCores, batch-sharded).

out = sqrt(gx^2 + gy^2), gx/gy = 3x3 depthwise convs (zero-padded) of
x [16, 64, 256, 256] fp32.

Per-core layout (2 batches x 64 ch = 128 images of 256x256):
  - image rows on partitions, two 128-row halves side by side in the free dim
  - vertical 3-taps as banded-matrix matmuls on TensorE (fp16 in, fp32 psum)
  - horizontal taps folded into PSUM accumulation via output-shifted matmuls
  - PSUM evacuated by ScalarE as Square; GPSIMD adds gx^2+gy^2; ScalarE Sqrt
  - rows 127/128 of each image (cross-half seam) recomputed in one batched
    late pass over all images and scattered over the main output
"""

import os
import numpy as np
from contextlib import ExitStack

import concourse.bacc as bacc
import concourse.mybir as mybir
from concourse.bass_utils import run_bass_kernel_spmd
from concourse.tile import TileContext, add_dep_helper

F32 = mybir.dt.float32
F16 = mybir.dt.float16

N_CORES = 8
B, C, H, W = 16, 64, 256, 256
B_LOC = B // N_CORES          # 2 batches per core
N_IMG = B_LOC * C             # 128 images per core
HALF = H // 2                 # 128 rows per half
WG = W + 2                    # guarded width (258)
GROUP = int(os.environ.get("SOBEL_GROUP", "2"))   # images per tail group
HYBRID_EVERY = int(os.environ.get("SOBEL_HYBRID", "0"))  # 0 = off
DVESQ_EVERY = int(os.environ.get("SOBEL_DVESQ", "0"))    # 0 = off
FLUSH_DELAY = int(os.environ.get("SOBEL_FLUSH_DELAY", "1"))


def _tap_matrices(kern):
    """kern: [3,3]. For each horizontal tap t in {-1,0,+1} build the banded
    vertical matrix V_t[k, m] = kern[di, t+1] for k = m + di - 1 (clipped).
    Returns list of (tap, V) for taps whose column is nonzero."""
    out = []
    for t in (-1, 0, 1):
        col = kern[:, t + 1]
        if not np.any(col):
            continue
        V = np.zeros((HALF, HALF), dtype=np.float32)
        for di in range(3):
            w = float(col[di])
            if w == 0.0:
                continue
            for m in range(HALF):
                k = m + di - 1
                if 0 <= k < HALF:
                    V[k, m] = w
        out.append((t, V))
    return out


def _mm_plan(kx, ky):
    """Unique weight matrices + per-image matmul descriptors.

    Returns (mats, descs): mats = list of unique [128,128] fp32 matrices;
    descs = ordered list of (slot, bank, off, start, stop) with matmuls
    grouped by weight slot (LDWEIGHTS reuse) and start/stop flags set on
    the first/last matmul of each PSUM bank in emission order."""
    gx_taps = _tap_matrices(kx)
    gy_taps = _tap_matrices(ky)
    mats, keys = [], {}

    def slot_of(V):
        k = V.tobytes()
        if k not in keys:
            keys[k] = len(mats)
            mats.append(V)
        return keys[k]

    def finalize(raw):
        raw = sorted(raw, key=lambda d: (d[0], d[1]))
        seen_first, last_idx = set(), {}
        for j, (s, b, off) in enumerate(raw):
            last_idx[b] = j
        descs = []
        for j, (s, b, off) in enumerate(raw):
            start = b not in seen_first
            seen_first.add(b)
            descs.append((s, b, off, start, last_idx[b] == j))
        return descs

    raw = []
    for h in range(2):
        for bank, taps in ((h, gx_taps), (2 + h, gy_taps)):
            for t, V in taps:
                raw.append((slot_of(V), bank, 512 * bank + (2 - (t + 1))))
    descs = finalize(raw)

    # Hybrid "B-path" (gy via DVE/GPSIMD smooth of d = Vb x): only valid when
    # the gy taps have the separable Sobel structure v_-1 == v_+1, v_0 == 2v.
    descs_b = None
    tapmap = {t: V for t, V in gy_taps}
    if (set(tapmap) == {-1, 0, 1}
            and np.array_equal(tapmap[-1], tapmap[1])
            and np.array_equal(tapmap[0], 2 * tapmap[-1])):
        vb_slot = slot_of(tapmap[-1])
        raw_b = []
        for h in range(2):
            for t, V in gx_taps:
                raw_b.append((slot_of(V), h, 512 * h + (2 - (t + 1))))
            raw_b.append((vb_slot, 2 + h, 512 * (2 + h)))
        descs_b = finalize(raw_b)
    return mats, descs, descs_b


def _build(nc, kx, ky):
    """Trace the bass program. kx, ky: 3x3 numpy Sobel kernels."""
    x_d = nc.dram_tensor("x", [B_LOC, C, H, W], F32, kind="ExternalInput")
    w_d = nc.dram_tensor("wts", [5, HALF, HALF], F16, kind="ExternalInput")
    out_d = nc.dram_tensor("out", [B_LOC, C, H, W], F32, kind="ExternalOutput")

    _mats, mm_descs, mm_descs_b = _mm_plan(kx, ky)

    x_flat = x_d[:].rearrange("b c h w -> (b c) h w")
    out_flat = out_d[:].rearrange("b c h w -> (b c) h w")

    out_dmas = []

    with ExitStack() as ctx:
        tc = ctx.enter_context(TileContext(nc))
        wpool = ctx.enter_context(tc.tile_pool(name="wts", bufs=1))
        xpool = ctx.enter_context(tc.tile_pool(name="xin", bufs=8))
        x16pool = ctx.enter_context(tc.tile_pool(name="x16", bufs=8))
        pspool = ctx.enter_context(tc.tile_pool(name="ps", bufs=2, space="PSUM"))
        qpool = ctx.enter_context(tc.tile_pool(name="qg", bufs=int(os.environ.get("SOBEL_QBUFS", "3"))))
        mpool = ctx.enter_context(tc.tile_pool(name="mg", bufs=3))
        opool = ctx.enter_context(tc.tile_pool(name="og", bufs=3))
        spool = ctx.enter_context(tc.tile_pool(name="seam", bufs=1))
        dpool = ctx.enter_context(tc.tile_pool(name="dsb", bufs=2))
        cpool = ctx.enter_context(tc.tile_pool(name="gxc", bufs=2))
        gypool = ctx.enter_context(tc.tile_pool(name="gyb", bufs=2))

        wt = wpool.tile([HALF, 5 * HALF], F16)
        nc.sync.dma_start(
            wt[:].rearrange("k (n m) -> k n m", n=5),
            w_d[:].rearrange("n k m -> k n m"),
        )

        def flush_m(q_g, m_g, pair):
            # m = gx^2 + gy^2 for one image pair on DVE (idle engine)
            qq = q_g[:].rearrange("p (i s c) -> p i s c", i=GROUP, s=2)
            nc.vector.tensor_tensor(
                m_g[:].rearrange("p (i c) -> p i c", i=GROUP)[
                    :, 2 * pair:2 * pair + 2, :],
                qq[:, 2 * pair:2 * pair + 2, 0, :],
                qq[:, 2 * pair:2 * pair + 2, 1, :], mybir.AluOpType.add,
            )

        def flush_tail(g, m_g):
            # sqrt + store for a whole group. Emitted late so the sqrt never
            # head-of-line-blocks the PSUM-recycling squares in ACT's queue.
            o_g = opool.tile([128, GROUP * 512], F32)
            nc.scalar.activation(o_g[:], m_g[:], mybir.ActivationFunctionType.Sqrt)
            d = nc.sync.dma_start(
                out_flat[g * GROUP:(g + 1) * GROUP].rearrange(
                    "i (h p) w -> p i h w", p=128
                ),
                o_g[:].rearrange("p (i h w) -> p i h w", i=GROUP, h=2),
            )
            out_dmas.append(d)

        # ---- late seam pass, part 1: computation emitted as small steps
        # spread across the main loop so it soaks up idle engine time ----
        sx = spool.tile([128, 4 * WG], F32)   # rows 126..129, guarded
        sxv = sx[:].rearrange("p (r c) -> p r c", r=4)
        seam_steps = []

        def _seam_gather():
            nc.gpsimd.memset(sxv[:, :, 0:WG:WG - 1], 0.0)
            nc.sync.dma_start(
                sxv[:, :, 1:W + 1], x_flat[:, H // 2 - 2:H // 2 + 2, :]
            )

        seam_steps.append(_seam_gather)

        def vcomb(name, col):
            """v[r] = sum_di col[di] * x[r + di - 1] for output block rows
            1..2 (image rows 127, 128), guarded width."""
            t = spool.tile([128, 2 * WG], F32, tag=f"v_{name}")
            tv = t[:].rearrange("p (r c) -> p r c", r=2)
            up, ce, dn = sxv[:, 0:2, :], sxv[:, 1:3, :], sxv[:, 2:4, :]
            tmp = spool.tile([128, 2 * WG], F32, tag=f"vt_{name}")
            tmpv = tmp[:].rearrange("p (r c) -> p r c", r=2)

            def _s1():
                nc.vector.tensor_scalar(tmpv[:], up, float(col[0]), None,
                                        mybir.AluOpType.mult)

            def _s2():
                nc.vector.scalar_tensor_tensor(
                    tmpv[:], ce, float(col[1]), tmpv[:],
                    mybir.AluOpType.mult, mybir.AluOpType.add)

            def _s3():
                nc.vector.scalar_tensor_tensor(
                    tv[:], dn, float(col[2]), tmpv[:],
                    mybir.AluOpType.mult, mybir.AluOpType.add)

            seam_steps.extend([_s1, _s2, _s3])
            return tv

        def hcomb(name, vs):
            """sum_t vs[t] shifted by t over data cols -> [128, 2, W]"""
            ot = spool.tile([128, 2 * W], F32, tag=f"h_{name}")
            otv = ot[:].rearrange("p (r c) -> p r c", r=2)
            items = sorted(vs.items())
            acc = None
            for i, (t, tv) in enumerate(items):
                sh = tv[:, :, 1 + t:1 + t + W]
                if acc is None:
                    if len(items) == 1:
                        seam_steps.append(
                            lambda o=otv, s=sh: nc.vector.tensor_copy(o[:], s))
                    acc = sh
                elif i == len(items) - 1:
                    seam_steps.append(
                        lambda o=otv, a=acc, s=sh:
                        nc.vector.tensor_tensor(o[:], a, s, mybir.AluOpType.add))
                else:
                    t2 = spool.tile([128, 2 * W], F32, tag=f"ha_{name}_{i}")
                    t2v = t2[:].rearrange("p (r c) -> p r c", r=2)
                    seam_steps.append(
                        lambda o=t2v, a=acc, s=sh:
                        nc.vector.tensor_tensor(o[:], a, s, mybir.AluOpType.add))
                    acc = t2v[:]
            return otv

        kxc = [[float(kx[di, t]) for di in range(3)] for t in range(3)]
        kyc = [[float(ky[di, t]) for di in range(3)] for t in range(3)]
        vgx = {t: vcomb(f"gx{t}", kxc[t + 1]) for t in (-1, 0, 1)
               if any(kxc[t + 1])}
        vgy = {t: vcomb(f"gy{t}", kyc[t + 1]) for t in (-1, 0, 1)
               if any(kyc[t + 1])}
        gxs = hcomb("gx", vgx)
        gys = hcomb("gy", vgy)
        q1s = spool.tile([128, 2 * W], F32)
        q2s = spool.tile([128, 2 * W], F32)
        ms = spool.tile([128, 2 * W], F32)
        os_ = spool.tile([128, 2 * W], F32)
        seam_steps.append(lambda: nc.scalar.activation(
            q1s[:], gxs, mybir.ActivationFunctionType.Square))
        seam_steps.append(lambda: nc.scalar.activation(
            q2s[:], gys, mybir.ActivationFunctionType.Square))
        seam_steps.append(lambda: nc.vector.tensor_tensor(
            ms[:], q1s[:], q2s[:], mybir.AluOpType.add))
        seam_steps.append(lambda: nc.scalar.activation(
            os_[:], ms[:], mybir.ActivationFunctionType.Sqrt))

        n_groups = N_IMG // GROUP
        pend = []
        for g in range(n_groups):
            q_g = qpool.tile([128, GROUP * 1024], F32)
            m_g = mpool.tile([128, GROUP * 512], F32)
            for gi in range(GROUP):
                img = g * GROUP + gi
                xin = xpool.tile([128, 2 * W], F32)
                nc.sync.dma_start(
                    xin[:].rearrange("p (h w) -> p h w", h=2),
                    x_flat[img].rearrange("(h p) w -> p h w", p=128),
                )
                x16 = x16pool.tile([128, 2 * WG], F16)
                x16v = x16[:].rearrange("p (h c) -> p h c", h=2)
                # zero the 4 guard columns (robust to slot rotation), then
                # convert the data columns fp32 -> fp16 on DVE
                nc.gpsimd.memset(x16v[:, :, 0:WG:WG - 1], 0.0)
                nc.vector.tensor_copy(
                    x16v[:, :, 1:W + 1],
                    xin[:].rearrange("p (h w) -> p h w", h=2),
                )
                # 4 PSUM banks: gx-h0 | gx-h1 | gy-h0 | gy-h1 (A path)
                # or gx-h0 | gx-h1 | d-h0 | d-h1 (B path: gy on DVE/GPSIMD)
                use_b = (mm_descs_b is not None and HYBRID_EVERY > 0
                         and img % HYBRID_EVERY == 0)
                ps = pspool.tile([128, 2048], F32)
                for wslot, b, off, start, stop in (
                        mm_descs_b if use_b else mm_descs):
                    nc.tensor.matmul(
                        ps[:, off:off + WG],
                        wt[:, wslot * HALF:(wslot + 1) * HALF],
                        x16[:, (b % 2) * WG:((b % 2) + 1) * WG],
                        start=start,
                        stop=stop,
                        skip_group_check=True,
                    )
                psb = ps[:].rearrange("p (b c) -> p b c", b=4)
                qv = q_g[:].rearrange("p (i b c) -> p (i b) c", i=GROUP, b=4)
                use_c = (not use_b and DVESQ_EVERY > 0
                         and img % DVESQ_EVERY == DVESQ_EVERY - 1)
                if use_c:
                    # gy^2 on ScalarE; gx evacuated + squared on DVE
                    nc.scalar.activation(
                        qv[:, gi * 4 + 2:gi * 4 + 4, :], psb[:, 2:4, 2:W + 2],
                        mybir.ActivationFunctionType.Square,
                    )
                    gxc = cpool.tile([128, 2 * W], F32)
                    gxv = gxc[:].rearrange("p (h c) -> p h c", h=2)
                    nc.vector.tensor_copy(gxv[:], psb[:, 0:2, 2:W + 2])
                    nc.vector.tensor_tensor(
                        qv[:, gi * 4:gi * 4 + 2, :], gxv[:], gxv[:],
                        mybir.AluOpType.mult)
                elif not use_b:
                    # q = (gx|gy)^2, all 4 banks in one ScalarE op
                    nc.scalar.activation(
                        qv[:, gi * 4:(gi + 1) * 4, :], psb[:, :, 2:W + 2],
                        mybir.ActivationFunctionType.Square,
                    )
                else:
                    # gx^2 on ScalarE (banks 0-1 only)
                    nc.scalar.activation(
                        qv[:, gi * 4:gi * 4 + 2, :], psb[:, 0:2, 2:W + 2],
                        mybir.ActivationFunctionType.Square,
                    )
                    # d -> SBUF (with guard cols), u = d_l + d_r on GPSIMD,
                    # gy = 2d + u on DVE, gy^2 into q_g on GPSIMD
                    dsb = dpool.tile([128, 2 * WG], F32)
                    dv = dsb[:].rearrange("p (h c) -> p h c", h=2)
                    nc.vector.tensor_copy(dv[:], psb[:, 2:4, 0:WG])
                    u = gypool.tile([128, 2 * W], F32, tag="u")
                    uv = u[:].rearrange("p (h c) -> p h c", h=2)
                    nc.gpsimd.tensor_tensor(
                        uv[:], dv[:, :, 0:W], dv[:, :, 2:W + 2],
                        mybir.AluOpType.add)
                    gy = gypool.tile([128, 2 * W], F32, tag="gy")
                    gyv = gy[:].rearrange("p (h c) -> p h c", h=2)
                    nc.vector.scalar_tensor_tensor(
                        gyv[:], dv[:, :, 1:W + 1], 2.0, uv[:],
                        mybir.AluOpType.mult, mybir.AluOpType.add)
                    nc.gpsimd.tensor_tensor(
                        qv[:, gi * 4 + 2:gi * 4 + 4, :], gyv[:], gyv[:],
                        mybir.AluOpType.mult)
                if gi % 2 == 1:
                    flush_m(q_g, m_g, gi // 2)
            pend.append((g, m_g))
            if len(pend) > FLUSH_DELAY:
                flush_tail(*pend.pop(0))
            if g >= 3 and seam_steps:
                seam_steps.pop(0)()
        while pend:
            flush_tail(*pend.pop(0))
        while seam_steps:
            seam_steps.pop(0)()

        seam_dma = nc.sync.dma_start(
            out_flat[:, H // 2 - 1:H // 2 + 1, :],
            os_[:].rearrange("p (r c) -> p r c", r=2),
        )
        # the seam scatter must land after the bulk output DMAs
        for d in out_dmas:
            try:
                add_dep_helper(seam_dma.ins, d.ins, reason="seam after bulk out")
            except Exception:
                pass
    return nc


def _make_weights(kx, ky):
    mats, _descs, _descs_b = _mm_plan(kx, ky)
    w = np.zeros((5, HALF, HALF), dtype=np.float16)
    for i, V in enumerate(mats):
        w[i] = V.astype(np.float16)
    return w


def kernel(x, sobel_x, sobel_y):
    x = np.asarray(x)
    kx = np.asarray(sobel_x).reshape(3, 3).astype(np.float32)
    ky = np.asarray(sobel_y).reshape(3, 3).astype(np.float32)

    nc = bacc.Bacc()
    _build(nc, kx, ky)
    nc.compile()

    wts = _make_weights(kx, ky)
    in_maps = [
        {"x": np.ascontiguousarray(x[i * B_LOC:(i + 1) * B_LOC]), "wts": wts}
        for i in range(N_CORES)
    ]
    kw = {}
    if os.environ.get("BASS_SOBEL_TRACE"):
        kw = {"trace": True}
    res = run_bass_kernel_spmd(nc, in_maps, core_ids=list(range(N_CORES)), **kw)
    global LAST_RESULTS
    LAST_RESULTS = res
    return np.concatenate([r["out"] for r in res.results], axis=0)


LAST_RESULTS = None

